# revision 16
# baseline (speedup 1.0000x reference)
"""Trainium2 Bass kernel for out = x @ W.T + b  (x:[8192,1024] f32, W:[1024,1024] f32, b:[1024] f32).

Data-parallel over batch across 8 NeuronCores: each core computes a
[1024,1024] @ [1024,1024]^T matmul + bias for its 1024-row batch shard.

Host-side prep (inside kernel(), not on device): shard x along batch,
pre-transpose x and W so the contraction dim (in_f) lands on SBUF
partitions with fully-contiguous per-partition DMA reads, and cast to
fp16 (rel err ~2.5e-4 against the 2e-2 gate).

Schedule (per core), tuned so the PE's 128-matmul stream (65536 PE
cycles = 27.3 us @ 2.4 GHz — the compute roofline) runs gap-free:

  - Inputs ride BOTH HWDGE rings in parallel: x ko-slabs on the SP ring,
    W chunks + bias on the ACT ring, so neither stream queues behind the
    other and the first matmul can start as soon as one x-slab and one
    W chunk have landed (~2.5 us).
  - First output half runs ko-OUTER (for ko: for t) with 8 live PSUM
    banks: each 8-matmul group consumes just one x-slab [128,8,128] and
    one W chunk [128,512], matching the DMA arrival order, so the PE
    never waits on a tile that is still in flight.
  - Second half runs t-outer (for t: for ko), closing one output tile
    per 1.7 us for a smooth DVE-add + store drain.
  - The last tile is split into four 128-col quarters so the final
    matmul->bias-add->store->completion chain (the serial tail) is
    ~1.3 us shorter than with a full 512-col tile.
  - Outputs are stored as fp16 (host upcasts) halving store traffic.
  - A short burst of dummy matmuls on a zeroed tile at t=0 keeps the PE
    busy while the first DMAs land, so the HAM clock-gate's ~3.4 us
    busy window elapses during the DMA ramp and the real stream runs at
    full clock (2.4 GHz) almost immediately.
"""

import os

import numpy as np

import concourse.bass as bass
import concourse.mybir as mybir
import concourse.tile as tile
from concourse import bacc
from concourse.bass_utils import run_bass_kernel_spmd

N_CORES = 8
B, IN_F, OUT_F = 8192, 1024, 1024
B_SHARD = B // N_CORES          # 1024 batch rows per core
P = 128                         # SBUF partitions
KO = IN_F // P                  # 8 contraction subtiles
NT = B_SHARD // P               # 8 batch tiles per core
NO = 2                          # 2 output column halves of 512
OW = OUT_F // NO                # 512 (one PSUM bank of fp32)
NQ = 4                          # last tile split into 4 col-quarters

MODE = os.environ.get("BASS_KERNEL_MODE", "f16")
N_WARMUP = int(os.environ.get("BASS_KERNEL_WARMUP", "30"))

_nc_cache = {}


def _build(mode):
    assert mode == "f16", mode
    f32 = mybir.dt.float32
    f16 = mybir.dt.float16

    nc = bacc.Bacc("TRN2", target_bir_lowering=False)

    # DRAM layouts are host-packed so every DMA is contiguous per partition:
    #   xt[ki, ko, t, bi]  = x_shard[t*128+bi, ko*128+ki]   (ko-slab major)
    #   wt[ki, ot, ko, oi] = W[ot*512+oi, ko*128+ki]
    #   bias[p, o]         = b[o]  (host-replicated across partitions)
    xt_d = nc.dram_tensor("xt", [P, KO, NT, P], f16, kind="ExternalInput")
    wt_d = nc.dram_tensor("wt", [P, NO, KO, OW], f16, kind="ExternalInput")
    bias_d = nc.dram_tensor("bias", [P, OUT_F], f32, kind="ExternalInput")
    out_d = nc.dram_tensor("out", [B_SHARD, OUT_F], f16, kind="ExternalOutput")

    with tile.TileContext(nc) as tc:
        with (
            tc.tile_pool(name="singles", bufs=1) as singles,
            tc.tile_pool(name="wpool", bufs=NO) as wpool,
            tc.tile_pool(name="xpool", bufs=1) as xpool,
            tc.tile_pool(name="opool", bufs=8) as opool,
            tc.tile_pool(name="psums", bufs=8, space="PSUM") as psums,
        ):
            bias_sb = singles.tile([P, OUT_F], f32)
            warm_sb = singles.tile([P, P], f16)

            w_tiles = [
                wpool.tile([P, KO, OW], f16, name=f"w_{ot}", tag="w_sb")
                for ot in range(NO)
            ]
            x_all = xpool.tile([P, KO, NT, P], f16, name="x_all", tag="x_sb")

            # PE warm-up: dummy matmuls on a zeroed tile, no DMA dependency.
            # High priority so the memset is the Pool engine's first op and
            # the PE starts right after the NEFF preamble. The dummies
            # bridge the PE until the first real matmul's inputs land, so
            # the HAM clock-gate's busy window (and the cost model's
            # p-state ramp) elapses during the DMA ramp and the real stream
            # runs at full clock with no idle gap.
            warm_ps = psums.tile([P, OW], f32, name="warm_ps", tag="ps")
            with tc.high_priority():
                nc.gpsimd.memset(warm_sb[:], 0)
                for _ in range(N_WARMUP):
                    nc.tensor.matmul(
                        warm_ps[:, 0:P], warm_sb[:], warm_sb[:],
                        start=True, stop=True,
                    )

            # Input DMAs in strict consumption order, alternating rings:
            # one x ko-slab + one W ko-chunk per 8-matmul group, so arrival
            # granularity matches consumption (~1.7us per pair). The Tile
            # scheduler has only 8 DMA completion lanes and a DMA's slot is
            # held ~4us (issue+transfer+semaphore), so issues 9+ wait for a
            # predecessor's completion; alternating x/w spreads those waits
            # evenly and every pair still lands ahead of its matmul group.
            nc.sync.dma_start(out=x_all[:, 0, 0:4], in_=xt_d[:, 0, 0:4])
            nc.scalar.dma_start(out=w_tiles[0][:, 0:1], in_=wt_d[:, 0, 0:1])
            nc.sync.dma_start(out=x_all[:, 0, 4:8], in_=xt_d[:, 0, 4:8])
            nc.scalar.dma_start(out=w_tiles[0][:, 1:2], in_=wt_d[:, 0, 1:2])
            for ko in range(1, KO):
                nc.sync.dma_start(out=x_all[:, ko], in_=xt_d[:, ko])
                if ko < KO - 1:
                    nc.scalar.dma_start(
                        out=w_tiles[0][:, ko + 1:ko + 2],
                        in_=wt_d[:, 0, ko + 1:ko + 2],
                    )
                elif ko == KO - 1:
                    nc.scalar.dma_start(
                        out=w_tiles[1][:, 0:4], in_=wt_d[:, 1, 0:4]
                    )
            nc.scalar.dma_start(out=w_tiles[1][:, 4:8], in_=wt_d[:, 1, 4:8])
            nc.scalar.dma_start(out=bias_sb[:], in_=bias_d[:])

            # Output half 0: ko-outer over 8 live PSUM banks. Group ko needs
            # only x-slab ko + W chunk ko — one DMA each ahead of the PE.
            # The bias add + store for tile t are issued right after its
            # closing (stop) matmul so each gets its own PE semaphore tick
            # and drains while the remaining tiles still accumulate.
            ps0 = [
                psums.tile([P, OW], f32, name=f"ps0_{t}", tag="ps")
                for t in range(NT)
            ]
            for ko in range(KO):
                for t in range(NT):
                    nc.tensor.matmul(
                        ps0[t][:],
                        x_all[:, ko, t],
                        w_tiles[0][:, ko],
                        start=(ko == 0),
                        stop=(ko == KO - 1),
                    )
                    if ko == KO - 1:
                        o = opool.tile([P, OW], f16, name=f"o0_{t}", tag="o_sb")
                        nc.vector.tensor_add(o[:], ps0[t][:], bias_sb[:, 0:OW])
                        nc.sync.dma_start(
                            out=out_d[t * P:(t + 1) * P, 0:OW], in_=o[:]
                        )

            # Output half 1: t-outer, one tile closes per group; the final
            # tile is split into quarters to shorten the serial tail.
            for t in range(NT):
                if t < NT - 1:
                    ps = psums.tile([P, OW], f32, name=f"ps1_{t}", tag="ps")
                    for ko in range(KO):
                        nc.tensor.matmul(
                            ps[:],
                            x_all[:, ko, t],
                            w_tiles[1][:, ko],
                            start=(ko == 0),
                            stop=(ko == KO - 1),
                        )
                    o = opool.tile([P, OW], f16, name=f"o1_{t}", tag="o_sb")
                    nc.vector.tensor_add(o[:], ps[:], bias_sb[:, OW:OUT_F])
                    nc.sync.dma_start(
                        out=out_d[t * P:(t + 1) * P, OW:OUT_F], in_=o[:]
                    )
                else:
                    # Uneven 320+192 split: the big piece's add+store drain
                    # while the PE finishes the small one, so the
                    # post-last-matmul chain is one short DVE add + one
                    # store. The big piece stores via the (idle) ACT ring so
                    # the final store never queues behind it on SP. Sizes
                    # balance the DVE: the 320-add clears the engine just as
                    # the 192-piece's closing matmul lands.
                    for c0, c1 in ((OW, OW + 320), (OW + 320, OUT_F)):
                        cw = c1 - c0
                        ps = psums.tile(
                            [P, cw], f32, name=f"ps1_{t}_{c0}", tag="ps"
                        )
                        for ko in range(KO):
                            nc.tensor.matmul(
                                ps[:],
                                x_all[:, ko, t],
                                w_tiles[1][:, ko, c0 - OW:c1 - OW],
                                start=(ko == 0),
                                stop=(ko == KO - 1),
                            )
                        o = opool.tile(
                            [P, cw], f16, name=f"o1_{t}_{c0}", tag="o_sb"
                        )
                        nc.vector.tensor_add(o[:], ps[:], bias_sb[:, c0:c1])
                        ring = nc.scalar if c1 != OUT_F else nc.sync
                        ring.dma_start(
                            out=out_d[t * P:(t + 1) * P, c0:c1], in_=o[:]
                        )
    nc.compile()
    return nc


def _get_nc(mode):
    if mode not in _nc_cache:
        _nc_cache[mode] = _build(mode)
    return _nc_cache[mode]


def _pack(x, W, b, mode="f16"):
    """Shard + retile host-side. Returns in_maps for the 8 cores."""
    x = np.asarray(x, dtype=np.float32)
    W = np.asarray(W, dtype=np.float32)
    b = np.asarray(b, dtype=np.float32)

    # [c, t, bi, ko, ki] -> [c, ki, ko, t, bi]
    xs = x.reshape(N_CORES, NT, P, KO, P).transpose(0, 4, 3, 1, 2)
    # [ot, oi, ko, ki] -> [ki, ot, ko, oi]
    ws = W.reshape(NO, OW, KO, P).transpose(3, 0, 2, 1)
    bias = np.ascontiguousarray(
        np.broadcast_to(b.reshape(1, OUT_F), (P, OUT_F))
    )

    xt = np.ascontiguousarray(xs).astype(np.float16)
    wt = np.ascontiguousarray(ws).astype(np.float16)
    return [{"xt": xt[c], "wt": wt, "bias": bias} for c in range(N_CORES)]


def _run(in_maps, mode="f16", **kwargs):
    nc = _get_nc("f16")
    return run_bass_kernel_spmd(nc, in_maps, core_ids=list(range(N_CORES)), **kwargs)


def kernel(x, W, b):
    res = _run(_pack(x, W, b))
    out = np.concatenate([r["out"] for r in res.results], axis=0)
    return np.ascontiguousarray(out, dtype=np.float32)


# revision 18
# speedup vs baseline: 1.0533x; 1.0533x over previous
"""Trainium2 Bass kernel for out = x @ W.T + b  (x:[8192,1024] f32, W:[1024,1024] f32, b:[1024] f32).

Data-parallel over batch across 8 NeuronCores: each core computes a
[1024,1024] @ [1024,1024]^T matmul + bias for its 1024-row batch shard.

Host-side prep (inside kernel(), not on device): shard x along batch,
pre-transpose x and W so the contraction dim (in_f) lands on SBUF
partitions with fully-contiguous per-partition DMA reads, and cast to
fp16 (rel err ~2.5e-4 against the 2e-2 gate).

Schedule (per core), tuned so the PE's 128-matmul stream (65536 PE
cycles = 27.3 us @ 2.4 GHz — the compute roofline) runs gap-free:

  - Inputs ride BOTH HWDGE rings in parallel: x ko-slabs on the SP ring,
    W chunks + bias on the ACT ring, so neither stream queues behind the
    other and the first matmul can start as soon as one x-slab and one
    W chunk have landed (~2.5 us).
  - First output half runs ko-OUTER (for ko: for t) with 8 live PSUM
    banks: each 8-matmul group consumes just one x-slab [128,8,128] and
    one W chunk [128,512], matching the DMA arrival order, so the PE
    never waits on a tile that is still in flight.
  - Second half runs t-outer (for t: for ko), closing one output tile
    per 1.7 us for a smooth DVE-add + store drain.
  - The last tile is split into four 128-col quarters so the final
    matmul->bias-add->store->completion chain (the serial tail) is
    ~1.3 us shorter than with a full 512-col tile.
  - Outputs are stored as fp16 (host upcasts) halving store traffic.
  - A short burst of dummy matmuls on a zeroed tile at t=0 keeps the PE
    busy while the first DMAs land, so the HAM clock-gate's ~3.4 us
    busy window elapses during the DMA ramp and the real stream runs at
    full clock (2.4 GHz) almost immediately.
"""

import os

import numpy as np

import concourse.bass as bass
import concourse.mybir as mybir
import concourse.tile as tile
from concourse import bacc
from concourse.bass_utils import run_bass_kernel_spmd

N_CORES = 8
B, IN_F, OUT_F = 8192, 1024, 1024
B_SHARD = B // N_CORES          # 1024 batch rows per core
P = 128                         # SBUF partitions
KO = IN_F // P                  # 8 contraction subtiles
NT = B_SHARD // P               # 8 batch tiles per core
NO = 2                          # 2 output column halves of 512
OW = OUT_F // NO                # 512 (one PSUM bank of fp32)
NQ = 4                          # last tile split into 4 col-quarters

MODE = os.environ.get("BASS_KERNEL_MODE", "f16")
N_WARMUP = int(os.environ.get("BASS_KERNEL_WARMUP", "30"))

_nc_cache = {}


def _build(mode):
    assert mode == "f16", mode
    f32 = mybir.dt.float32
    f16 = mybir.dt.float16

    nc = bacc.Bacc("TRN2", target_bir_lowering=False)

    # DRAM layouts are host-packed so every DMA is contiguous per partition.
    # Each ko row of xw packs the W half-0 chunk AND the x slab one matmul
    # group consumes, so ONE DMA delivers a whole group's inputs:
    #   xw[ki, ko, 0:512]        = W[ko*128+ki, 0:512]      (half-0 chunk)
    #   xw[ki, ko, 512+t*128+bi] = x_shard[t*128+bi, ko*128+ki]
    #   w1[ki, ko, oi]           = W[512+oi, ko*128+ki]     (half 1)
    #   bias[p, o]               = b[o]  (host-replicated across partitions)
    XWW = OW + NT * P              # 1536 elements per ko row
    xw_d = nc.dram_tensor("xw", [P, KO, XWW], f16, kind="ExternalInput")
    w1_d = nc.dram_tensor("w1", [P, KO, OW], f16, kind="ExternalInput")
    bias_d = nc.dram_tensor("bias", [P, OUT_F], f32, kind="ExternalInput")
    out_d = nc.dram_tensor("out", [B_SHARD, OUT_F], f16, kind="ExternalOutput")

    with tile.TileContext(nc) as tc:
        with (
            tc.tile_pool(name="singles", bufs=1) as singles,
            tc.tile_pool(name="wpool", bufs=NO) as wpool,
            tc.tile_pool(name="xpool", bufs=1) as xpool,
            tc.tile_pool(name="opool", bufs=8) as opool,
            tc.tile_pool(name="psums", bufs=8, space="PSUM") as psums,
        ):
            bias_sb = singles.tile([P, OUT_F], f32)
            warm_sb = singles.tile([P, P], f16)

            w1_sb = wpool.tile([P, KO, OW], f16, name="w1", tag="w_sb")
            xw_sb = xpool.tile([P, KO, XWW], f16, name="xw", tag="x_sb")

            # PE warm-up: dummy matmuls on a zeroed tile, no DMA dependency.
            # High priority so the memset is the Pool engine's first op and
            # the PE starts right after the NEFF preamble. The dummies
            # bridge the PE until the first real matmul's inputs land, so
            # the HAM clock-gate's busy window (and the cost model's
            # p-state ramp) elapses during the DMA ramp and the real stream
            # runs at full clock with no idle gap.
            warm_ps = psums.tile([P, OW], f32, name="warm_ps", tag="ps")
            with tc.high_priority():
                nc.gpsimd.memset(warm_sb[:], 0)
                for _ in range(N_WARMUP):
                    nc.tensor.matmul(
                        warm_ps[:, 0:P], warm_sb[:], warm_sb[:],
                        start=True, stop=True,
                    )

            # Input DMAs in strict consumption order: one packed xw DMA
            # feeds each 8-matmul group (arrival granularity = consumption
            # granularity), and the first 8 DMAs exactly fill the Tile
            # scheduler's 8 DMA completion lanes, so no issue ever blocks
            # on a lane slot mid-phase. The first group's row is split in
            # two halves across both rings so its W chunk + first x tiles
            # land in parallel as early as possible.
            nc.sync.dma_start(out=xw_sb[:, 0, 0:896], in_=xw_d[:, 0, 0:896])
            nc.scalar.dma_start(
                out=xw_sb[:, 0, 896:XWW], in_=xw_d[:, 0, 896:XWW]
            )
            for ko in range(1, KO):
                ring = nc.sync if ko % 2 else nc.scalar
                ring.dma_start(out=xw_sb[:, ko], in_=xw_d[:, ko])
            nc.scalar.dma_start(out=w1_sb[:, 0:4], in_=w1_d[:, 0:4])
            nc.sync.dma_start(out=w1_sb[:, 4:8], in_=w1_d[:, 4:8])
            nc.scalar.dma_start(out=bias_sb[:], in_=bias_d[:])

            # Output half 0: ko-outer over 8 live PSUM banks. Group ko needs
            # only x-slab ko + W chunk ko — one DMA each ahead of the PE.
            # The bias add + store for tile t are issued right after its
            # closing (stop) matmul so each gets its own PE semaphore tick
            # and drains while the remaining tiles still accumulate.
            ps0 = [
                psums.tile([P, OW], f32, name=f"ps0_{t}", tag="ps")
                for t in range(NT)
            ]
            for ko in range(KO):
                for t in range(NT):
                    nc.tensor.matmul(
                        ps0[t][:],
                        xw_sb[:, ko, OW + t * P:OW + (t + 1) * P],
                        xw_sb[:, ko, 0:OW],
                        start=(ko == 0),
                        stop=(ko == KO - 1),
                    )
                    if ko == KO - 1:
                        o = opool.tile([P, OW], f16, name=f"o0_{t}", tag="o_sb")
                        nc.vector.tensor_add(o[:], ps0[t][:], bias_sb[:, 0:OW])
                        nc.sync.dma_start(
                            out=out_d[t * P:(t + 1) * P, 0:OW], in_=o[:]
                        )

            # Output half 1: t-outer, one tile closes per group; the final
            # tile is split into quarters to shorten the serial tail.
            for t in range(NT):
                if t < NT - 1:
                    ps = psums.tile([P, OW], f32, name=f"ps1_{t}", tag="ps")
                    for ko in range(KO):
                        nc.tensor.matmul(
                            ps[:],
                            xw_sb[:, ko, OW + t * P:OW + (t + 1) * P],
                            w1_sb[:, ko],
                            start=(ko == 0),
                            stop=(ko == KO - 1),
                        )
                    o = opool.tile([P, OW], f16, name=f"o1_{t}", tag="o_sb")
                    nc.vector.tensor_add(o[:], ps[:], bias_sb[:, OW:OUT_F])
                    nc.sync.dma_start(
                        out=out_d[t * P:(t + 1) * P, OW:OUT_F], in_=o[:]
                    )
                else:
                    # Uneven 320+192 split: the big piece's add+store drain
                    # while the PE finishes the small one, so the
                    # post-last-matmul chain is one short DVE add + one
                    # store. The big piece stores via the (idle) ACT ring so
                    # the final store never queues behind it on SP. Sizes
                    # balance the DVE: the 320-add clears the engine just as
                    # the 192-piece's closing matmul lands.
                    for c0, c1 in ((OW, OW + 320), (OW + 320, OUT_F)):
                        cw = c1 - c0
                        ps = psums.tile(
                            [P, cw], f32, name=f"ps1_{t}_{c0}", tag="ps"
                        )
                        for ko in range(KO):
                            nc.tensor.matmul(
                                ps[:],
                                xw_sb[:, ko, OW + t * P:OW + (t + 1) * P],
                                w1_sb[:, ko, c0 - OW:c1 - OW],
                                start=(ko == 0),
                                stop=(ko == KO - 1),
                            )
                        o = opool.tile(
                            [P, cw], f16, name=f"o1_{t}_{c0}", tag="o_sb"
                        )
                        nc.vector.tensor_add(o[:], ps[:], bias_sb[:, c0:c1])
                        ring = nc.scalar if c1 != OUT_F else nc.sync
                        ring.dma_start(
                            out=out_d[t * P:(t + 1) * P, c0:c1], in_=o[:]
                        )
    nc.compile()
    return nc


def _get_nc(mode):
    if mode not in _nc_cache:
        _nc_cache[mode] = _build(mode)
    return _nc_cache[mode]


def _pack(x, W, b, mode="f16"):
    """Shard + retile host-side. Returns in_maps for the 8 cores."""
    x = np.asarray(x, dtype=np.float32)
    W = np.asarray(W, dtype=np.float32)
    b = np.asarray(b, dtype=np.float32)

    # [c, t, bi, ko, ki] -> [c, ki, ko, t*bi]
    xs = (
        x.reshape(N_CORES, NT, P, KO, P)
        .transpose(0, 4, 3, 1, 2)
        .reshape(N_CORES, P, KO, NT * P)
    )
    # [ot, oi, ko, ki] -> [ki, ot, ko, oi]
    ws = W.reshape(NO, OW, KO, P).transpose(3, 0, 2, 1)
    bias = np.ascontiguousarray(
        np.broadcast_to(b.reshape(1, OUT_F), (P, OUT_F))
    )

    xs16 = xs.astype(np.float16)
    ws16 = np.ascontiguousarray(ws).astype(np.float16)
    # xw[c, ki, ko, :] = [W half-0 chunk (512) | x slab (1024)]
    w0rep = np.broadcast_to(ws16[:, 0][None], (N_CORES, P, KO, OW))
    xw = np.ascontiguousarray(np.concatenate([w0rep, xs16], axis=-1))
    w1 = np.ascontiguousarray(ws16[:, 1])
    return [{"xw": xw[c], "w1": w1, "bias": bias} for c in range(N_CORES)]


def _run(in_maps, mode="f16", **kwargs):
    nc = _get_nc("f16")
    return run_bass_kernel_spmd(nc, in_maps, core_ids=list(range(N_CORES)), **kwargs)


def kernel(x, W, b):
    res = _run(_pack(x, W, b))
    out = np.concatenate([r["out"] for r in res.results], axis=0)
    return np.ascontiguousarray(out, dtype=np.float32)


# revision 19
# speedup vs baseline: 1.0697x; 1.0156x over previous
"""Trainium2 Bass kernel for out = x @ W.T + b  (x:[8192,1024] f32, W:[1024,1024] f32, b:[1024] f32).

Data-parallel over batch across 8 NeuronCores: each core computes a
[1024,1024] @ [1024,1024]^T matmul + bias for its 1024-row batch shard.

Host-side prep (inside kernel(), not on device): shard x along batch,
pre-transpose x and W so the contraction dim (in_f) lands on SBUF
partitions with fully-contiguous per-partition DMA reads, and cast to
fp16 (rel err ~2.5e-4 against the 2e-2 gate).

Schedule (per core), tuned so the PE's 128-matmul stream (65536 PE
cycles = 27.3 us @ 2.4 GHz — the compute roofline) runs gap-free:

  - Inputs ride BOTH HWDGE rings in parallel: x ko-slabs on the SP ring,
    W chunks + bias on the ACT ring, so neither stream queues behind the
    other and the first matmul can start as soon as one x-slab and one
    W chunk have landed (~2.5 us).
  - First output half runs ko-OUTER (for ko: for t) with 8 live PSUM
    banks: each 8-matmul group consumes just one x-slab [128,8,128] and
    one W chunk [128,512], matching the DMA arrival order, so the PE
    never waits on a tile that is still in flight.
  - Second half runs t-outer (for t: for ko), closing one output tile
    per 1.7 us for a smooth DVE-add + store drain.
  - The last tile is split into four 128-col quarters so the final
    matmul->bias-add->store->completion chain (the serial tail) is
    ~1.3 us shorter than with a full 512-col tile.
  - Outputs are stored as fp16 (host upcasts) halving store traffic.
  - A short burst of dummy matmuls on a zeroed tile at t=0 keeps the PE
    busy while the first DMAs land, so the HAM clock-gate's ~3.4 us
    busy window elapses during the DMA ramp and the real stream runs at
    full clock (2.4 GHz) almost immediately.
"""

import os

import numpy as np

import concourse.bass as bass
import concourse.mybir as mybir
import concourse.tile as tile
from concourse import bacc
from concourse.bass_utils import run_bass_kernel_spmd

N_CORES = 8
B, IN_F, OUT_F = 8192, 1024, 1024
B_SHARD = B // N_CORES          # 1024 batch rows per core
P = 128                         # SBUF partitions
KO = IN_F // P                  # 8 contraction subtiles
NT = B_SHARD // P               # 8 batch tiles per core
NO = 2                          # 2 output column halves of 512
OW = OUT_F // NO                # 512 (one PSUM bank of fp32)
NQ = 4                          # last tile split into 4 col-quarters

MODE = os.environ.get("BASS_KERNEL_MODE", "f16")
N_WARMUP = int(os.environ.get("BASS_KERNEL_WARMUP", "44"))

_nc_cache = {}


def _build(mode):
    assert mode == "f16", mode
    f32 = mybir.dt.float32
    f16 = mybir.dt.float16

    nc = bacc.Bacc("TRN2", target_bir_lowering=False)

    # DRAM layouts are host-packed so every DMA is contiguous per partition.
    # Each ko row of xw packs the W half-0 chunk AND the x slab one matmul
    # group consumes, so ONE DMA delivers a whole group's inputs:
    #   xw[ki, ko, 0:512]        = W[ko*128+ki, 0:512]      (half-0 chunk)
    #   xw[ki, ko, 512+t*128+bi] = x_shard[t*128+bi, ko*128+ki]
    #   w1[ki, ko, oi]           = W[512+oi, ko*128+ki]     (half 1)
    #   bias[p, o]               = b[o]  (host-replicated across partitions)
    XWW = OW + NT * P              # 1536 elements per ko row
    xw_d = nc.dram_tensor("xw", [P, KO, XWW], f16, kind="ExternalInput")
    w1_d = nc.dram_tensor("w1", [P, KO, OW], f16, kind="ExternalInput")
    bias_d = nc.dram_tensor("bias", [P, OUT_F], f32, kind="ExternalInput")
    out_d = nc.dram_tensor("out", [B_SHARD, OUT_F], f16, kind="ExternalOutput")

    with tile.TileContext(nc) as tc:
        with (
            tc.tile_pool(name="singles", bufs=1) as singles,
            tc.tile_pool(name="wpool", bufs=NO) as wpool,
            tc.tile_pool(name="xpool", bufs=1) as xpool,
            tc.tile_pool(name="opool", bufs=8) as opool,
            tc.tile_pool(name="psums", bufs=8, space="PSUM") as psums,
        ):
            bias_sb = singles.tile([P, OUT_F], f32)
            warm_sb = singles.tile([P, P], f16)

            w1_sb = wpool.tile([P, KO, OW], f16, name="w1", tag="w_sb")
            xw_sb = xpool.tile([P, KO, XWW], f16, name="xw", tag="x_sb")

            # PE warm-up: dummy matmuls on a zeroed tile, no DMA dependency.
            # High priority so the memset is the Pool engine's first op and
            # the PE starts right after the NEFF preamble. The dummies
            # bridge the PE until the first real matmul's inputs land, so
            # the HAM clock-gate's busy window (and the cost model's
            # p-state ramp) elapses during the DMA ramp and the real stream
            # runs at full clock with no idle gap.
            warm_ps = psums.tile([P, OW], f32, name="warm_ps", tag="ps")
            with tc.high_priority():
                nc.gpsimd.memset(warm_sb[:], 0)
                for _ in range(N_WARMUP):
                    nc.tensor.matmul(
                        warm_ps[:, 0:P], warm_sb[:], warm_sb[:],
                        start=True, stop=True,
                    )

            # Input DMAs in strict consumption order: one packed xw DMA
            # feeds each 8-matmul group (arrival granularity = consumption
            # granularity), and the first 8 DMAs exactly fill the Tile
            # scheduler's 8 DMA completion lanes, so no issue ever blocks
            # on a lane slot mid-phase. The first group's row is split in
            # two halves across both rings so its W chunk + first x tiles
            # land in parallel as early as possible.
            nc.sync.dma_start(out=xw_sb[:, 0, 0:896], in_=xw_d[:, 0, 0:896])
            nc.scalar.dma_start(
                out=xw_sb[:, 0, 896:XWW], in_=xw_d[:, 0, 896:XWW]
            )
            for ko in range(1, KO):
                ring = nc.sync if ko % 2 else nc.scalar
                ring.dma_start(out=xw_sb[:, ko], in_=xw_d[:, ko])
            nc.scalar.dma_start(out=w1_sb[:, 0:4], in_=w1_d[:, 0:4])
            nc.sync.dma_start(out=w1_sb[:, 4:8], in_=w1_d[:, 4:8])
            nc.scalar.dma_start(out=bias_sb[:], in_=bias_d[:])

            # Output half 0: ko-outer over 8 live PSUM banks. Group ko needs
            # only x-slab ko + W chunk ko — one DMA each ahead of the PE.
            # The bias add + store for tile t are issued right after its
            # closing (stop) matmul so each gets its own PE semaphore tick
            # and drains while the remaining tiles still accumulate.
            ps0 = [
                psums.tile([P, OW], f32, name=f"ps0_{t}", tag="ps")
                for t in range(NT)
            ]
            for ko in range(KO):
                for t in range(NT):
                    nc.tensor.matmul(
                        ps0[t][:],
                        xw_sb[:, ko, OW + t * P:OW + (t + 1) * P],
                        xw_sb[:, ko, 0:OW],
                        start=(ko == 0),
                        stop=(ko == KO - 1),
                    )
                    if ko == KO - 1:
                        o = opool.tile([P, OW], f16, name=f"o0_{t}", tag="o_sb")
                        nc.vector.tensor_add(o[:], ps0[t][:], bias_sb[:, 0:OW])
                        nc.sync.dma_start(
                            out=out_d[t * P:(t + 1) * P, 0:OW], in_=o[:]
                        )

            # Output half 1: t-outer, one tile closes per group; the final
            # tile is split into quarters to shorten the serial tail.
            for t in range(NT):
                if t < NT - 1:
                    ps = psums.tile([P, OW], f32, name=f"ps1_{t}", tag="ps")
                    for ko in range(KO):
                        nc.tensor.matmul(
                            ps[:],
                            xw_sb[:, ko, OW + t * P:OW + (t + 1) * P],
                            w1_sb[:, ko],
                            start=(ko == 0),
                            stop=(ko == KO - 1),
                        )
                    o = opool.tile([P, OW], f16, name=f"o1_{t}", tag="o_sb")
                    nc.vector.tensor_add(o[:], ps[:], bias_sb[:, OW:OUT_F])
                    nc.sync.dma_start(
                        out=out_d[t * P:(t + 1) * P, OW:OUT_F], in_=o[:]
                    )
                else:
                    # Uneven 320+192 split: the big piece's add+store drain
                    # while the PE finishes the small one, so the
                    # post-last-matmul chain is one short DVE add + one
                    # store. The big piece stores via the (idle) ACT ring so
                    # the final store never queues behind it on SP. Sizes
                    # balance the DVE: the 320-add clears the engine just as
                    # the 192-piece's closing matmul lands.
                    for c0, c1 in ((OW, OW + 320), (OW + 320, OUT_F)):
                        cw = c1 - c0
                        ps = psums.tile(
                            [P, cw], f32, name=f"ps1_{t}_{c0}", tag="ps"
                        )
                        for ko in range(KO):
                            nc.tensor.matmul(
                                ps[:],
                                xw_sb[:, ko, OW + t * P:OW + (t + 1) * P],
                                w1_sb[:, ko, c0 - OW:c1 - OW],
                                start=(ko == 0),
                                stop=(ko == KO - 1),
                            )
                        o = opool.tile(
                            [P, cw], f16, name=f"o1_{t}_{c0}", tag="o_sb"
                        )
                        nc.vector.tensor_add(o[:], ps[:], bias_sb[:, c0:c1])
                        ring = nc.scalar if c1 != OUT_F else nc.sync
                        ring.dma_start(
                            out=out_d[t * P:(t + 1) * P, c0:c1], in_=o[:]
                        )
    nc.compile()
    return nc


def _get_nc(mode):
    if mode not in _nc_cache:
        _nc_cache[mode] = _build(mode)
    return _nc_cache[mode]


def _pack(x, W, b, mode="f16"):
    """Shard + retile host-side. Returns in_maps for the 8 cores."""
    x = np.asarray(x, dtype=np.float32)
    W = np.asarray(W, dtype=np.float32)
    b = np.asarray(b, dtype=np.float32)

    # [c, t, bi, ko, ki] -> [c, ki, ko, t*bi]
    xs = (
        x.reshape(N_CORES, NT, P, KO, P)
        .transpose(0, 4, 3, 1, 2)
        .reshape(N_CORES, P, KO, NT * P)
    )
    # [ot, oi, ko, ki] -> [ki, ot, ko, oi]
    ws = W.reshape(NO, OW, KO, P).transpose(3, 0, 2, 1)
    bias = np.ascontiguousarray(
        np.broadcast_to(b.reshape(1, OUT_F), (P, OUT_F))
    )

    xs16 = xs.astype(np.float16)
    ws16 = np.ascontiguousarray(ws).astype(np.float16)
    # xw[c, ki, ko, :] = [W half-0 chunk (512) | x slab (1024)]
    w0rep = np.broadcast_to(ws16[:, 0][None], (N_CORES, P, KO, OW))
    xw = np.ascontiguousarray(np.concatenate([w0rep, xs16], axis=-1))
    w1 = np.ascontiguousarray(ws16[:, 1])
    return [{"xw": xw[c], "w1": w1, "bias": bias} for c in range(N_CORES)]


def _run(in_maps, mode="f16", **kwargs):
    nc = _get_nc("f16")
    return run_bass_kernel_spmd(nc, in_maps, core_ids=list(range(N_CORES)), **kwargs)


def kernel(x, W, b):
    res = _run(_pack(x, W, b))
    out = np.concatenate([r["out"] for r in res.results], axis=0)
    return np.ascontiguousarray(out, dtype=np.float32)
